# revision 16
# baseline (speedup 1.0000x reference)
"""Multi-head self-attention (B=2, S=2048, D=1024, H=16) on 8 trn2 cores.

Sharding: core c = b*4 + g  (b = batch, g = head-group of 4 heads).
Each core computes, for its batch b and heads 4g..4g+3:
  Qt = (Wq_g^T x_b^T + bq_g),  Kt likewise   -> [256, 2048] feature-major
  V  = x_b Wv_g                               -> [2048, 256] seq-major (no bias)
  scoresT[k,q] = sum_d Kt[d,k] Qt[d,q]        (per 128-key chunk)
  expT = exp(0.125 * scoresT)                 (ACT, straight from PSUM)
  ctxT/rowsum via PV matmul with V||ones      -> psum [65, 512]
  ctxT normalized by 1/rowsum (DMA partition-broadcast + DVE mult)
  y_partial = ctx_g @ Wo_g                    -> [2048, 1024]
Host: Y[b] = sum_g y_partial + (bo + bv @ Wo).
"""

import sys

sys.path.insert(0, "/opt/trn_rl_repo")

import numpy as np

import concourse.bass as bass
import concourse.mybir as mybir
import concourse.tile as tile

F32 = mybir.dt.float32
F32R = mybir.dt.float32r
AF = mybir.ActivationFunctionType

D = 1024          # d_model
S = 2048          # sequence length
HPC = 4           # heads per core
DK = 64           # head dim
E = HPC * DK      # 256 features per core
N_CORES = 8





_ENGINE_OPS = {
    "InstMatmult", "InstActivation", "InstTensorCopy", "InstTensorTensor",
    "InstReciprocal", "InstTensorReduce", "InstMemset", "InstIota",
    "InstTensorScalarPtr", "InstTranspose", "InstLdweights",
    "InstDMACopy", "InstDrain", "InstNoOp",
}


def _legalize_matmul_waits(nc):
    """walrus allows at most 1 sync wait on engine compute instructions; Tile
    sometimes emits more. Move the excess onto EventSemaphore instructions
    (cap 2 each) placed immediately before in same-engine program order."""
    for f in nc.m.functions:
        for bb in f.blocks:
            out = []
            changed = False
            for i in bb.instructions:
                si = getattr(i, "sync_info", None)
                if (
                    type(i).__name__ in _ENGINE_OPS
                    and si is not None
                    and si.on_wait
                    and len(si.on_wait) > 1
                ):
                    waits = list(si.on_wait)
                    excess, keep = waits[:-1], waits[-1:]
                    for c in range(0, len(excess), 2):
                        ev = mybir.InstEventSemaphore(
                            name=f"{i.name}-mmw{c}", ins=[], outs=[]
                        )
                        ev.engine = i.engine
                        ev.sync_info = mybir.SyncInfo(
                            on_wait=excess[c:c + 2], on_update=[]
                        )
                        out.append(ev)
                    i.sync_info = mybir.SyncInfo(
                        on_wait=keep, on_update=list(si.on_update)
                    )
                    changed = True
                out.append(i)
            if changed:
                bb.instructions = out


def build_nc():
    nc = bass.Bass()

    xt = nc.dram_tensor("xt", [D, S], F32R, kind="ExternalInput")
    wq = nc.dram_tensor("wq", [D, E], F32R, kind="ExternalInput")
    wk = nc.dram_tensor("wk", [D, E], F32R, kind="ExternalInput")
    wv = nc.dram_tensor("wv", [D, E], F32R, kind="ExternalInput")
    wo = nc.dram_tensor("wo", [E, D], F32R, kind="ExternalInput")
    bq = nc.dram_tensor("bq", [E], F32, kind="ExternalInput")
    bk = nc.dram_tensor("bk", [E], F32, kind="ExternalInput")
    y = nc.dram_tensor("y", [S, D], F32, kind="ExternalOutput")

    KT = D // 128     # 8 k-tiles over d_model
    QC = S // 512     # 4 q-chunks of 512
    SC = S // 128     # 16 seq chunks of 128 (key chunks)
    ET = E // 128     # 2 feature tiles

    with tile.TileContext(nc) as tc:
        with tc.tile_pool(name="persist", bufs=1) as pp:
            # ---- persistent tiles ----
            qt_sb = [pp.tile([128, S], F32R, tag=f"qt{t}", name=f"qt{t}") for t in range(ET)]
            kt_sb = [pp.tile([128, S], F32R, tag=f"kt{t}", name=f"kt{t}") for t in range(ET)]
            # V with a ones column per head: [128, h, 65]
            v_sb = [pp.tile([128, HPC, DK + 1], F32R, tag=f"v{s}", name=f"v{s}") for s in range(SC)]
            ctx_sb = [pp.tile([128, S], F32R, tag=f"ctx{t}", name=f"ctx{t}") for t in range(ET)]
            wo_sb = [pp.tile([128, D], F32R, tag=f"wo{t}", name=f"wo{t}") for t in range(ET)]
            bq_sb = pp.tile([128, ET], F32, tag="bq")
            bk_sb = pp.tile([128, ET], F32, tag="bk")

            nc.sync.dma_start(bq_sb, bq.rearrange("(t p) -> p t", p=128))
            nc.sync.dma_start(bk_sb, bk.rearrange("(t p) -> p t", p=128))
            for t in range(ET):
                nc.sync.dma_start(wo_sb[t], wo[t * 128:(t + 1) * 128, :])
            ones_sb = pp.tile([128, HPC], F32, tag="ones")
            nc.vector.memset(ones_sb, 1.0)
            for s in range(SC):
                nc.vector.tensor_copy(
                    v_sb[s][:, :, DK:DK + 1], ones_sb[:, :, None]
                )

            # ---- stage A: projections ----
            with (
                tc.tile_pool(name="stageA", bufs=1) as pa,
                tc.tile_pool(name="psA", bufs=6, space="PSUM") as psA,
            ):
                xt_sb = [pa.tile([128, S], F32R, tag=f"xt{k}", name=f"xt{k}") for k in range(KT)]
                wq_sb = pa.tile([128, KT, E], F32R, tag="wq")
                wk_sb = pa.tile([128, KT, E], F32R, tag="wk")
                wv_sb = pa.tile([128, KT, E], F32R, tag="wv")
                for k in range(KT):
                    nc.sync.dma_start(wq_sb[:, k, :], wq[k * 128:(k + 1) * 128, :])
                    nc.sync.dma_start(wk_sb[:, k, :], wk[k * 128:(k + 1) * 128, :])
                    nc.sync.dma_start(wv_sb[:, k, :], wv[k * 128:(k + 1) * 128, :])
                    nc.sync.dma_start(xt_sb[k], xt[k * 128:(k + 1) * 128, :])

                # Qt/Kt: feature-major [e, s];  out = W_tile^T @ xt
                for t in range(ET):
                    for qc in range(QC):
                        for w_sb, b_sb, out in (
                            (wq_sb, bq_sb, qt_sb),
                            (wk_sb, bk_sb, kt_sb),
                        ):
                            ps = psA.tile([128, 512], F32, tag="proj")
                            for k in range(KT):
                                nc.tensor.matmul(
                                    ps,
                                    w_sb[:, k, t * 128:(t + 1) * 128],
                                    xt_sb[k][:, qc * 512:(qc + 1) * 512],
                                    start=(k == 0),
                                    stop=(k == KT - 1),
                                )
                            nc.scalar.activation(
                                out[t][:, qc * 512:(qc + 1) * 512], ps,
                                AF.Identity, bias=b_sb[:, t:t + 1],
                            )

                # V: seq-major [s, e];  out = xt_tile^T @ wv
                for s in range(SC):
                    ps = psA.tile([128, E], F32, tag="proj", name=f"vps{s}")
                    for k in range(KT):
                        nc.tensor.matmul(
                            ps,
                            xt_sb[k][:, s * 128:(s + 1) * 128],
                            wv_sb[:, k, :],
                            start=(k == 0),
                            stop=(k == KT - 1),
                        )
                    nc.vector.tensor_copy(
                        v_sb[s][:, :, 0:DK],
                        ps.rearrange("p (h d) -> p h d", h=HPC),
                    )

            # ---- stage B: attention ----
            with (
                tc.tile_pool(name="stageB", bufs=3) as pb,
                tc.tile_pool(name="dramB", bufs=3, space="DRAM") as dramB,
                tc.tile_pool(name="psS", bufs=2, space="PSUM") as psS,
                tc.tile_pool(name="psC", bufs=4, space="PSUM") as psC,
            ):
                for h in range(HPC):
                    t, off = h // 2, (h % 2) * 64
                    for qh in range(2):               # q halves of 1024
                        ctx_ps = []
                        for j in range(2):            # ctx psum [65, 512] per q-chunk
                            ctx_ps.append(psC.tile([DK + 1, 512], F32, tag="ctx", name=f"ctxps{h}_{qh}_{j}"))
                        for kc in range(SC):
                            sc_ps = psS.tile([128, 1024], F32, tag="sc")
                            for j in range(2):
                                nc.tensor.matmul(
                                    sc_ps[:, j * 512:(j + 1) * 512],
                                    kt_sb[t][off:off + 64,
                                               kc * 128:(kc + 1) * 128],
                                    qt_sb[t][off:off + 64,
                                               qh * 1024 + j * 512:
                                               qh * 1024 + (j + 1) * 512],
                                    start=True, stop=True,
                                )
                            ex = pb.tile([128, 1024], F32R, tag="ex")
                            nc.scalar.activation(ex, sc_ps, AF.Exp, scale=0.125)
                            for j in range(2):
                                nc.tensor.matmul(
                                    ctx_ps[j],
                                    v_sb[kc][:, h, :],
                                    ex[:, j * 512:(j + 1) * 512],
                                    start=(kc == 0),
                                    stop=(kc == SC - 1),
                                )
                        # normalize: ctx[:64]/rowsum (row 64)
                        recip = pb.tile([1, 1024], F32, tag="recip")
                        rb = pb.tile([64, 1024], F32, tag="rb")
                        for j in range(2):
                            nc.vector.reciprocal(
                                recip[:, j * 512:(j + 1) * 512],
                                ctx_ps[j][DK:DK + 1, :],
                            )
                        recip_dr = dramB.tile([1, 1024], F32, tag="recip_dr",
                                              name=f"recipdr{h}_{qh}")
                        nc.sync.dma_start(recip_dr, recip)
                        nc.sync.dma_start(rb, recip_dr.to_broadcast([64, 1024]))
                        for j in range(2):
                            nc.vector.tensor_mul(
                                ctx_sb[t][off:off + 64,
                                          qh * 1024 + j * 512:
                                          qh * 1024 + (j + 1) * 512],
                                ctx_ps[j][0:DK, :],
                                rb[:, j * 512:(j + 1) * 512],
                            )

            # ---- stage C: output projection ----
            with (
                tc.tile_pool(name="stageC", bufs=3) as pc,
                tc.tile_pool(name="psY", bufs=2, space="PSUM") as psY,
            ):
                for qt in range(SC):
                    for n in range(2):
                        yp = psY.tile([128, 512], F32, tag="y")
                        for t in range(ET):
                            nc.tensor.matmul(
                                yp,
                                ctx_sb[t][:, qt * 128:(qt + 1) * 128],
                                wo_sb[t][:, n * 512:(n + 1) * 512],
                                start=(t == 0),
                                stop=(t == ET - 1),
                            )
                        ys = pc.tile([128, 512], F32, tag="ys")
                        nc.vector.tensor_copy(ys, yp)
                        nc.sync.dma_start(
                            y[qt * 128:(qt + 1) * 128, n * 512:(n + 1) * 512], ys
                        )
    _legalize_matmul_waits(nc)
    return nc


_NC_CACHE = None


def _get_nc():
    global _NC_CACHE
    if _NC_CACHE is None:
        _NC_CACHE = build_nc()
    return _NC_CACHE


def make_in_maps(inputs):
    x = np.asarray(inputs["x"], dtype=np.float32)
    Wq = np.asarray(inputs["Wq"], dtype=np.float32)
    Wk = np.asarray(inputs["Wk"], dtype=np.float32)
    Wv = np.asarray(inputs["Wv"], dtype=np.float32)
    Wo = np.asarray(inputs["Wo"], dtype=np.float32)
    bq = np.asarray(inputs["bq"], dtype=np.float32)
    bk = np.asarray(inputs["bk"], dtype=np.float32)

    in_maps = []
    for c in range(N_CORES):
        b, g = c // 4, c % 4
        sl = slice(g * E, (g + 1) * E)
        in_maps.append({
            "xt": np.ascontiguousarray(x[b].T),
            "wq": np.ascontiguousarray(Wq[:, sl]),
            "wk": np.ascontiguousarray(Wk[:, sl]),
            "wv": np.ascontiguousarray(Wv[:, sl]),
            "wo": np.ascontiguousarray(Wo[sl, :]),
            "bq": np.ascontiguousarray(bq[sl]),
            "bk": np.ascontiguousarray(bk[sl]),
        })
    return in_maps


def kernel(x, Wq, bq, Wk, bk, Wv, bv, Wo, bo):
    from concourse.bass_utils import run_bass_kernel_spmd

    x = np.asarray(x, dtype=np.float32)
    Wv = np.asarray(Wv, dtype=np.float32)
    Wo = np.asarray(Wo, dtype=np.float32)
    bv = np.asarray(bv, dtype=np.float32)
    bo = np.asarray(bo, dtype=np.float32)

    B = x.shape[0]
    nc = _get_nc()
    in_maps = make_in_maps({
        "x": x, "Wq": Wq, "Wk": Wk, "Wv": Wv, "Wo": Wo, "bq": bq, "bk": bk,
    })

    res = run_bass_kernel_spmd(nc, in_maps, core_ids=list(range(N_CORES)))

    bias_total = bo + bv @ Wo  # [D]
    out = np.zeros((B, S, D), dtype=np.float32)
    for c in range(N_CORES):
        out[c // 4] += res.results[c]["y"]
    out += bias_total[None, None, :]
    return out


# revision 20
# speedup vs baseline: 1.1269x; 1.1269x over previous
"""Multi-head self-attention (B=2, S=2048, D=1024, H=16) on 8 trn2 cores.

Sharding: core c = b*4 + g  (b = batch, g = head-group of 4 heads).
Each core computes, for its batch b and heads 4g..4g+3:
  Qt = (Wq_g^T x_b^T + bq_g),  Kt likewise   -> [256, 2048] feature-major
  V  = x_b Wv_g                               -> [2048, 256] seq-major (no bias)
  scoresT[k,q] = sum_d Kt[d,k] Qt[d,q]        (per 128-key chunk)
  expT = exp(0.125 * scoresT)                 (ACT, straight from PSUM)
  ctxT/rowsum via PV matmul with V||ones      -> psum [65, 512]
  ctxT normalized by 1/rowsum (DMA partition-broadcast + DVE mult)
  y_partial = ctx_g @ Wo_g                    -> [2048, 1024]
Host: Y[b] = sum_g y_partial + (bo + bv @ Wo).
"""

import sys

sys.path.insert(0, "/opt/trn_rl_repo")

import numpy as np

import concourse.bass as bass
import concourse.mybir as mybir
import concourse.tile as tile

F32 = mybir.dt.float32
F32R = mybir.dt.float32r
AF = mybir.ActivationFunctionType

D = 1024          # d_model
S = 2048          # sequence length
HPC = 4           # heads per core
DK = 64           # head dim
E = HPC * DK      # 256 features per core
N_CORES = 8





_ENGINE_OPS = {
    "InstMatmult", "InstActivation", "InstTensorCopy", "InstTensorTensor",
    "InstReciprocal", "InstTensorReduce", "InstMemset", "InstIota",
    "InstTensorScalarPtr", "InstTranspose", "InstLdweights",
    "InstDMACopy", "InstDrain", "InstNoOp",
}


def _legalize_matmul_waits(nc):
    """walrus allows at most 1 sync wait on engine compute instructions; Tile
    sometimes emits more. Move the excess onto EventSemaphore instructions
    (cap 2 each) placed immediately before in same-engine program order."""
    for f in nc.m.functions:
        for bb in f.blocks:
            out = []
            changed = False
            for i in bb.instructions:
                si = getattr(i, "sync_info", None)
                if (
                    type(i).__name__ in _ENGINE_OPS
                    and si is not None
                    and si.on_wait
                    and len(si.on_wait) > 1
                ):
                    waits = list(si.on_wait)
                    excess, keep = waits[:-1], waits[-1:]
                    for c in range(0, len(excess), 2):
                        ev = mybir.InstEventSemaphore(
                            name=f"{i.name}-mmw{c}", ins=[], outs=[]
                        )
                        ev.engine = i.engine
                        ev.sync_info = mybir.SyncInfo(
                            on_wait=excess[c:c + 2], on_update=[]
                        )
                        out.append(ev)
                    i.sync_info = mybir.SyncInfo(
                        on_wait=keep, on_update=list(si.on_update)
                    )
                    changed = True
                out.append(i)
            if changed:
                bb.instructions = out


def build_nc():
    nc = bass.Bass()

    xt = nc.dram_tensor("xt", [D, S], F32R, kind="ExternalInput")
    wq = nc.dram_tensor("wq", [D, E], F32R, kind="ExternalInput")
    wk = nc.dram_tensor("wk", [D, E], F32R, kind="ExternalInput")
    wv = nc.dram_tensor("wv", [D, E], F32R, kind="ExternalInput")
    wo = nc.dram_tensor("wo", [E, D], F32R, kind="ExternalInput")
    bq = nc.dram_tensor("bq", [E], F32, kind="ExternalInput")
    bk = nc.dram_tensor("bk", [E], F32, kind="ExternalInput")
    y = nc.dram_tensor("y", [S, D], F32, kind="ExternalOutput")

    KT = D // 128     # 8 k-tiles over d_model
    QC = S // 512     # 4 q-chunks of 512
    SC = S // 128     # 16 seq chunks of 128 (key chunks)
    ET = E // 128     # 2 feature tiles

    with tile.TileContext(nc) as tc:
        with tc.tile_pool(name="persist", bufs=1) as pp:
            # ---- persistent tiles ----
            qt_sb = [pp.tile([128, S], F32R, tag=f"qt{t}", name=f"qt{t}") for t in range(ET)]
            kt_sb = [pp.tile([128, S], F32R, tag=f"kt{t}", name=f"kt{t}") for t in range(ET)]
            # V with a ones column per head: [128, h, 65]
            v_sb = [pp.tile([128, HPC, DK + 1], F32R, tag=f"v{s}", name=f"v{s}") for s in range(SC)]
            ctx_sb = [pp.tile([128, S], F32R, tag=f"ctx{t}", name=f"ctx{t}") for t in range(ET)]
            wo_sb = [pp.tile([128, D], F32R, tag=f"wo{t}", name=f"wo{t}") for t in range(ET)]
            bq_sb = pp.tile([128, ET], F32, tag="bq")
            bk_sb = pp.tile([128, ET], F32, tag="bk")

            nc.sync.dma_start(bq_sb, bq.rearrange("(t p) -> p t", p=128))
            nc.sync.dma_start(bk_sb, bk.rearrange("(t p) -> p t", p=128))
            for t in range(ET):
                nc.sync.dma_start(wo_sb[t], wo[t * 128:(t + 1) * 128, :])
            ones_sb = pp.tile([128, HPC], F32, tag="ones")
            nc.vector.memset(ones_sb, 1.0)
            for s in range(SC):
                nc.vector.tensor_copy(
                    v_sb[s][:, :, DK:DK + 1], ones_sb[:, :, None]
                )

            # ---- stage A: projections ----
            with (
                tc.tile_pool(name="stageA", bufs=1) as pa,
                tc.tile_pool(name="psA", bufs=8, space="PSUM") as psA,
            ):
                xt_sb = [pa.tile([128, S], F32R, tag=f"xt{k}", name=f"xt{k}") for k in range(KT)]
                wq_sb = pa.tile([128, KT, E], F32R, tag="wq")
                wk_sb = pa.tile([128, KT, E], F32R, tag="wk")
                wv_sb = pa.tile([128, KT, E], F32R, tag="wv")
                for k in range(KT):
                    nc.sync.dma_start(wq_sb[:, k, :], wq[k * 128:(k + 1) * 128, :])
                    nc.sync.dma_start(wk_sb[:, k, :], wk[k * 128:(k + 1) * 128, :])
                    nc.sync.dma_start(wv_sb[:, k, :], wv[k * 128:(k + 1) * 128, :])
                    nc.sync.dma_start(xt_sb[k], xt[k * 128:(k + 1) * 128, :])

                # Qt/Kt: feature-major [e, s];  out = W_tile^T @ xt.
                # k-outer with 8 resident psum accumulators so the first
                # matmuls start as soon as xt[0]/w[0] land.
                for w_sb, b_sb, out in (
                    (wq_sb, bq_sb, qt_sb),
                    (wk_sb, bk_sb, kt_sb),
                ):
                    pss = [
                        psA.tile([128, 512], F32, tag="proj",
                                 name=f"pp{id(w_sb)}_{t}_{qc}")
                        for t in range(ET) for qc in range(QC)
                    ]
                    for k in range(KT):
                        for t in range(ET):
                            for qc in range(QC):
                                nc.tensor.matmul(
                                    pss[t * QC + qc],
                                    w_sb[:, k, t * 128:(t + 1) * 128],
                                    xt_sb[k][:, qc * 512:(qc + 1) * 512],
                                    start=(k == 0),
                                    stop=(k == KT - 1),
                                )
                    for t in range(ET):
                        for qc in range(QC):
                            nc.scalar.activation(
                                out[t][:, qc * 512:(qc + 1) * 512],
                                pss[t * QC + qc],
                                AF.Identity, bias=b_sb[:, t:t + 1],
                            )

                # V: seq-major [s, e];  out = xt_tile^T @ wv
                for s in range(SC):
                    ps = psA.tile([128, E], F32, tag="proj", name=f"vps{s}")
                    for k in range(KT):
                        nc.tensor.matmul(
                            ps,
                            xt_sb[k][:, s * 128:(s + 1) * 128],
                            wv_sb[:, k, :],
                            start=(k == 0),
                            stop=(k == KT - 1),
                        )
                    nc.vector.tensor_copy(
                        v_sb[s][:, :, 0:DK],
                        ps.rearrange("p (h d) -> p h d", h=HPC),
                    )

            # ---- stage B: attention ----
            with (
                tc.tile_pool(name="stageB", bufs=3) as pb,
                tc.tile_pool(name="dramB", bufs=3, space="DRAM") as dramB,
                tc.tile_pool(name="psS", bufs=1, space="PSUM") as psS,
                tc.tile_pool(name="psC", bufs=4, space="PSUM") as psC,
            ):
                # Heads are processed as partition-pairs (A at rows 0:64,
                # B at rows 64:128 of the same Qt/Kt tile). Interleaving the
                # two heads keeps PE busy while ACT runs the other head's
                # exp, so the PE HAM clock stays at 2.4 GHz.
                for t in range(ET):                   # head pair (2t, 2t+1)
                    offs = (0, 64)
                    for qh in range(2):               # q halves of 1024
                        ctx_ps = {
                            (hp, j): psC.tile([DK + 1, 512], F32, tag="ctx",
                                              name=f"ctxps{t}_{qh}_{hp}_{j}")
                            for hp in range(2) for j in range(2)
                        }
                        for kc in range(SC):
                            sc = []
                            for hp in range(2):
                                sc_ps = psS.tile(
                                    [128, 1024], F32, tag=f"sc{hp}",
                                    name=f"sc{t}_{qh}_{hp}_{kc}",
                                )
                                off = offs[hp]
                                for j in range(2):
                                    nc.tensor.matmul(
                                        sc_ps[:, j * 512:(j + 1) * 512],
                                        kt_sb[t][off:off + 64,
                                                 kc * 128:(kc + 1) * 128],
                                        qt_sb[t][off:off + 64,
                                                 qh * 1024 + j * 512:
                                                 qh * 1024 + (j + 1) * 512],
                                        start=True, stop=True,
                                    )
                                sc.append(sc_ps)
                            ex = []
                            for hp in range(2):
                                e = pb.tile([128, 1024], F32R, tag=f"ex{hp}",
                                            name=f"ex{t}_{qh}_{hp}_{kc}")
                                nc.scalar.activation(e, sc[hp], AF.Exp,
                                                     scale=0.125)
                                ex.append(e)
                            for hp in range(2):
                                for j in range(2):
                                    nc.tensor.matmul(
                                        ctx_ps[hp, j],
                                        v_sb[kc][:, 2 * t + hp, :],
                                        ex[hp][:, j * 512:(j + 1) * 512],
                                        start=(kc == 0),
                                        stop=(kc == SC - 1),
                                    )
                        # normalize: ctx[:64]/rowsum (row 64)
                        for hp in range(2):
                            off = offs[hp]
                            recip = pb.tile([1, 1024], F32, tag="recip",
                                            name=f"recip{t}_{qh}_{hp}")
                            rb = pb.tile([64, 1024], F32, tag="rb",
                                         name=f"rb{t}_{qh}_{hp}")
                            for j in range(2):
                                nc.vector.reciprocal(
                                    recip[:, j * 512:(j + 1) * 512],
                                    ctx_ps[hp, j][DK:DK + 1, :],
                                )
                            recip_dr = dramB.tile(
                                [1, 1024], F32, tag="recip_dr",
                                name=f"recipdr{t}_{qh}_{hp}",
                            )
                            nc.sync.dma_start(recip_dr, recip)
                            nc.sync.dma_start(
                                rb, recip_dr.to_broadcast([64, 1024])
                            )
                            for j in range(2):
                                nc.vector.tensor_mul(
                                    ctx_sb[t][off:off + 64,
                                              qh * 1024 + j * 512:
                                              qh * 1024 + (j + 1) * 512],
                                    ctx_ps[hp, j][0:DK, :],
                                    rb[:, j * 512:(j + 1) * 512],
                                )

            # ---- stage C: output projection ----
            with (
                tc.tile_pool(name="stageC", bufs=3) as pc,
                tc.tile_pool(name="psY", bufs=2, space="PSUM") as psY,
            ):
                for qt in range(SC):
                    for n in range(2):
                        yp = psY.tile([128, 512], F32, tag="y")
                        for t in range(ET):
                            nc.tensor.matmul(
                                yp,
                                ctx_sb[t][:, qt * 128:(qt + 1) * 128],
                                wo_sb[t][:, n * 512:(n + 1) * 512],
                                start=(t == 0),
                                stop=(t == ET - 1),
                            )
                        ys = pc.tile([128, 512], F32, tag="ys")
                        nc.vector.tensor_copy(ys, yp)
                        nc.sync.dma_start(
                            y[qt * 128:(qt + 1) * 128, n * 512:(n + 1) * 512], ys
                        )
    _legalize_matmul_waits(nc)
    return nc


_NC_CACHE = None


def _get_nc():
    global _NC_CACHE
    if _NC_CACHE is None:
        _NC_CACHE = build_nc()
    return _NC_CACHE


def make_in_maps(inputs):
    x = np.asarray(inputs["x"], dtype=np.float32)
    Wq = np.asarray(inputs["Wq"], dtype=np.float32)
    Wk = np.asarray(inputs["Wk"], dtype=np.float32)
    Wv = np.asarray(inputs["Wv"], dtype=np.float32)
    Wo = np.asarray(inputs["Wo"], dtype=np.float32)
    bq = np.asarray(inputs["bq"], dtype=np.float32)
    bk = np.asarray(inputs["bk"], dtype=np.float32)

    in_maps = []
    for c in range(N_CORES):
        b, g = c // 4, c % 4
        sl = slice(g * E, (g + 1) * E)
        in_maps.append({
            "xt": np.ascontiguousarray(x[b].T),
            "wq": np.ascontiguousarray(Wq[:, sl]),
            "wk": np.ascontiguousarray(Wk[:, sl]),
            "wv": np.ascontiguousarray(Wv[:, sl]),
            "wo": np.ascontiguousarray(Wo[sl, :]),
            "bq": np.ascontiguousarray(bq[sl]),
            "bk": np.ascontiguousarray(bk[sl]),
        })
    return in_maps


def kernel(x, Wq, bq, Wk, bk, Wv, bv, Wo, bo):
    from concourse.bass_utils import run_bass_kernel_spmd

    x = np.asarray(x, dtype=np.float32)
    Wv = np.asarray(Wv, dtype=np.float32)
    Wo = np.asarray(Wo, dtype=np.float32)
    bv = np.asarray(bv, dtype=np.float32)
    bo = np.asarray(bo, dtype=np.float32)

    B = x.shape[0]
    nc = _get_nc()
    in_maps = make_in_maps({
        "x": x, "Wq": Wq, "Wk": Wk, "Wv": Wv, "Wo": Wo, "bq": bq, "bk": bk,
    })

    res = run_bass_kernel_spmd(nc, in_maps, core_ids=list(range(N_CORES)))

    bias_total = bo + bv @ Wo  # [D]
    out = np.zeros((B, S, D), dtype=np.float32)
    for c in range(N_CORES):
        out[c // 4] += res.results[c]["y"]
    out += bias_total[None, None, :]
    return out
